# revision 1
# baseline (speedup 1.0000x reference)
"""MoE MLP (E=32 experts, top-2, D=H=1024) on 8 Trainium2 NeuronCores.

Strategy (expert parallel, per sharding hint):
  * Host computes the (tiny) gate: softmax(x @ Wg), top-2, renormalized
    weights, and dispatches tokens per expert into capacity-padded blocks,
    transposed to [D, tokens] (features on SBUF partitions, tokens on the
    matmul moving/free dimension). This is the sharding/all-to-all step.
  * Each of the 8 cores owns 4 experts (W1/W2/b1/b2 shards) and computes
    GELU(x W1 + b1) W2 + b2 for its experts' token blocks.
  * Host combines with the top-2 gate weights (scatter-add).

Device kernel notes:
  * Weights are host-pre-tiled to [e, col_tile, partition, k_tile, 128] so
    each half-layer streams in as one fully-contiguous DMA chunk.
  * dma_start triggers cost ~0.4-0.8us serialized on the issuing engine's
    sequencer, so transfers are few and large, and triggers are spread
    across engine queues (sync=weights, vector=x, scalar=y, gpsimd=bias).
  * A short chain of dummy matmuls at kernel start warms the PE clock
    (HAM) while the first weight DMAs land.
"""

import os
import sys
import numpy as np

for _p in ("/root/.axon_site/_ro/trn_rl_repo", "/opt/trn_rl_repo"):
    if _p not in sys.path and os.path.isdir(_p):
        sys.path.append(_p)

E, D, H = 32, 1024, 1024
TOP_K = 2
N_CORES = 8
EPC = E // N_CORES  # experts per core
ND = D // 128       # d 128-tiles
NH = H // 128       # h 128-tiles

# weight dtype, activation dtype (must both be 16-bit or both 32-bit)
DT_W = os.environ.get("MOE_DT_W", "bfloat16")
DT_A = os.environ.get("MOE_DT_A", "bfloat16")
N_WARMUP_MM = int(os.environ.get("MOE_WARMUP", "20"))

LAST_EXEC_TIME_NS = None

_NC_CACHE = {}


def _build_nc(TCH, CW, dt_w_name, dt_a_name):
    import concourse.bass as bass  # noqa: F401
    import concourse.tile as tile
    from concourse import bacc, mybir
    from contextlib import ExitStack

    f32 = mybir.dt.float32
    dt_w = getattr(mybir.dt, dt_w_name)
    dt_a = getattr(mybir.dt, dt_a_name)
    C = TCH * CW

    nc = bacc.Bacc(
        "TRN2",
        target_bir_lowering=False,
        debug=False,
        enable_asserts=False,
        num_devices=N_CORES,
    )
    xT = nc.dram_tensor("xT", [D, EPC * C], dt_a, kind="ExternalInput").ap()
    # host-pre-tiled: w1[e, ht, p(=d_in), dt, hi], w2[e, dt, p(=h_in), ht, di]
    w1 = nc.dram_tensor("w1", [EPC, NH, 128, ND, 128], dt_w, kind="ExternalInput").ap()
    w2 = nc.dram_tensor("w2", [EPC, ND, 128, NH, 128], dt_w, kind="ExternalInput").ap()
    # host-pre-transposed biases: [p, e, col_tile]
    b1 = nc.dram_tensor("b1", [128, EPC, NH], f32, kind="ExternalInput").ap()
    b2 = nc.dram_tensor("b2", [128, EPC, ND], f32, kind="ExternalInput").ap()
    yT = nc.dram_tensor("yT", [D, EPC * C], f32, kind="ExternalOutput").ap()

    HNH = NH // 2  # half-layer column split
    HND = ND // 2
    # weight-pool lookahead: 4-byte weights are SBUF-tight
    WB = 3 if mybir.dt.size(dt_w) == 4 else 4

    with tile.TileContext(nc) as tc, ExitStack() as ctx:
        wpool = ctx.enter_context(tc.tile_pool(name="w", bufs=4))
        xpool = ctx.enter_context(tc.tile_pool(name="x", bufs=2))
        hpool = ctx.enter_context(tc.tile_pool(name="h", bufs=2 * NH))
        ypool = ctx.enter_context(tc.tile_pool(name="y", bufs=2))
        bpool = ctx.enter_context(tc.tile_pool(name="b", bufs=1))
        pp1 = ctx.enter_context(tc.tile_pool(name="ps1", bufs=3, space="PSUM"))
        pp2 = ctx.enter_context(tc.tile_pool(name="ps2", bufs=3, space="PSUM"))
        ppw = ctx.enter_context(tc.tile_pool(name="psw", bufs=1, space="PSUM"))

        # PE warm-up: dummy matmuls with no DMA dependency keep the PE
        # busy from t~0 so HAM un-throttles before the real matmuls.
        if N_WARMUP_MM:
            wu = bpool.tile([128, 512], mybir.dt.bfloat16, tag="wu")
            nc.vector.memset(wu[:], 0.0)
            wups = ppw.tile([128, 512], f32, tag="psw")
            for i in range(N_WARMUP_MM):
                nc.tensor.matmul(wups[:], wu[:, :128], wu[:],
                                 start=(i == 0), stop=(i == N_WARMUP_MM - 1))

        gelu = mybir.ActivationFunctionType.Gelu
        b1_sb = b2_sb = None
        for e in range(EPC):
            # tokens: one DMA per expert -> [p, (dt, tok)]
            xt = xpool.tile([128, ND * C], dt_a, tag="xt")
            nc.gpsimd.dma_start(
                out=xt[:].rearrange("p (dt t) -> p dt t", dt=ND),
                in_=xT[:, e * C:(e + 1) * C].rearrange("(dt p) t -> p dt t", p=128),
            )
            # weights: W1 in column chunks (quarters for the first expert so
            # compute starts on the first 512KB), in consumption order
            n_chunks = 4 if e == 0 else 2
            csz = NH // n_chunks
            w1h = []
            for half in range(n_chunks):
                wt = wpool.tile([128, csz * ND * 128], dt_w,
                                tag=f"w1c{n_chunks}",
                                bufs=(4 if n_chunks == 4 else WB))
                nc.sync.dma_start(
                    out=wt[:].rearrange("p (ht dt hi) -> p ht dt hi", ht=csz, dt=ND),
                    in_=w1[e, half * csz:(half + 1) * csz].rearrange(
                        "ht p dt hi -> p ht dt hi"),
                )
                w1h.append(wt)
            if b1_sb is None:
                b1_sb = bpool.tile([128, EPC * NH], f32, tag="b1")
                b2_sb = bpool.tile([128, EPC * ND], f32, tag="b2")
                nc.gpsimd.dma_start(
                    out=b1_sb[:].rearrange("p (e ht) -> p e ht", e=EPC), in_=b1[:])
                nc.gpsimd.dma_start(
                    out=b2_sb[:].rearrange("p (e dt) -> p e dt", e=EPC), in_=b2[:])
            w2h = []
            for half in range(2):
                wt = wpool.tile([128, HND * NH * 128], dt_w, tag="w2c",
                                bufs=WB)
                nc.sync.dma_start(
                    out=wt[:].rearrange("p (dt ht di) -> p dt ht di", dt=HND, ht=NH),
                    in_=w2[e, half * HND:(half + 1) * HND].rearrange(
                        "dt p ht di -> p dt ht di"),
                )
                w2h.append(wt)

            for ch in range(TCH):
                hts = []
                for ht in range(NH):
                    wt = w1h[ht // csz]
                    hoff = (ht % csz) * ND * 128
                    ps = pp1.tile([128, CW], f32, tag="ps1")
                    for dt_i in range(ND):
                        nc.tensor.matmul(
                            ps[:],
                            wt[:, hoff + dt_i * 128: hoff + (dt_i + 1) * 128],
                            xt[:, dt_i * C + ch * CW: dt_i * C + (ch + 1) * CW],
                            start=(dt_i == 0),
                            stop=(dt_i == ND - 1),
                        )
                    hsb = hpool.tile([128, CW], dt_a, tag="ht")
                    nc.scalar.activation(
                        hsb[:], ps[:], gelu,
                        bias=b1_sb[:, e * NH + ht: e * NH + ht + 1],
                    )
                    hts.append(hsb)
                ysb = ypool.tile([128, ND * CW], f32, tag="yt")
                for dt_i in range(ND):
                    wt = w2h[dt_i // HND]
                    doff = (dt_i % HND) * NH * 128
                    ps2 = pp2.tile([128, CW], f32, tag="ps2")
                    for ht in range(NH):
                        nc.tensor.matmul(
                            ps2[:],
                            wt[:, doff + ht * 128: doff + (ht + 1) * 128],
                            hts[ht][:],
                            start=(ht == 0),
                            stop=(ht == NH - 1),
                        )
                    nc.vector.tensor_scalar_add(
                        ysb[:, dt_i * CW:(dt_i + 1) * CW], ps2[:],
                        b2_sb[:, e * ND + dt_i: e * ND + dt_i + 1],
                    )
                for half in range(2):
                    r0, r1 = half * HND * 128, (half + 1) * HND * 128
                    nc.scalar.dma_start(
                        out=yT[r0:r1, e * C + ch * CW: e * C + (ch + 1) * CW]
                        .rearrange("(dt p) t -> p dt t", p=128),
                        in_=ysb[:, half * HND * CW:(half + 1) * HND * CW]
                        .rearrange("p (dt t) -> p dt t", dt=HND),
                    )
    nc.compile()
    return nc


def _get_nc(TCH, CW, dt_w, dt_a):
    key = (TCH, CW, dt_w, dt_a)
    if key not in _NC_CACHE:
        _NC_CACHE[key] = _build_nc(TCH, CW, dt_w, dt_a)
    return _NC_CACHE[key]


def _np_dt(name):
    if name == "bfloat16":
        import ml_dtypes
        return np.dtype(ml_dtypes.bfloat16)
    return np.dtype(np.float32)


def _route(xf, Wg):
    """Replicates the reference gate exactly in f32 numpy."""
    logits = xf @ Wg                                     # [T, E]
    m = logits.max(-1, keepdims=True)
    ex = np.exp(logits - m)
    scores = ex / ex.sum(-1, keepdims=True)
    idx = np.argsort(-scores, axis=1, kind="stable")[:, :TOP_K]  # [T, k]
    tw = np.take_along_axis(scores, idx, 1)
    m2 = tw.max(-1, keepdims=True)
    e2 = np.exp(tw - m2)
    w = (e2 / e2.sum(-1, keepdims=True)).astype(np.float32)
    return idx.astype(np.int64), w


def kernel(x, Wg, W1, b1, W2, b2):
    global LAST_EXEC_TIME_NS
    from concourse import bass_utils

    dt_w, dt_a = DT_W, DT_A
    orig_shape = x.shape
    x = np.asarray(x, dtype=np.float32)
    Wg = np.asarray(Wg, dtype=np.float32)
    W1 = np.asarray(W1, dtype=np.float32)
    b1 = np.asarray(b1, dtype=np.float32)
    W2 = np.asarray(W2, dtype=np.float32)
    b2 = np.asarray(b2, dtype=np.float32)
    xf = np.ascontiguousarray(x.reshape(-1, D))
    T = xf.shape[0]

    idx, w = _route(xf, Wg)

    # ---- dispatch: per-expert capacity-padded token blocks
    flat_e = idx.reshape(-1)                 # [k*T]
    flat_t = np.repeat(np.arange(T), TOP_K)
    order = np.argsort(flat_e, kind="stable")
    counts = np.bincount(flat_e, minlength=E)
    maxc = int(counts.max())
    C = max(256, -(-maxc // 16) * 16)
    TCH = -(-C // 512)
    CW = -(-C // (TCH * 16)) * 16
    C = TCH * CW

    starts = np.zeros(E + 1, np.int64)
    starts[1:] = np.cumsum(counts)
    se = flat_e[order]
    pos = np.arange(TOP_K * T) - starts[se]
    core = se // EPC
    col = (se % EPC) * C + pos               # column in that core's xT
    tok = flat_t[order]

    gidx = np.zeros((N_CORES, EPC * C), np.int64)
    for c in range(N_CORES):
        msel = core == c
        gidx[c, col[msel]] = tok[msel]

    np_w = _np_dt(dt_w)
    np_a = _np_dt(dt_a)
    xf_a = xf.astype(np_a, copy=False)
    # pre-tile weights: w1 -> [e, ht, p(d_in), dt, hi], w2 -> [e, dt, p(h_in), ht, di]
    W1t = np.ascontiguousarray(
        W1.reshape(E, ND, 128, NH, 128).transpose(0, 3, 2, 1, 4).astype(np_w, copy=False))
    W2t = np.ascontiguousarray(
        W2.reshape(E, NH, 128, ND, 128).transpose(0, 3, 2, 1, 4).astype(np_w, copy=False))
    # pre-transpose biases to [p, e, col_tile]
    b1t = np.ascontiguousarray(b1.reshape(E, NH, 128).transpose(2, 0, 1))
    b2t = np.ascontiguousarray(b2.reshape(E, ND, 128).transpose(2, 0, 1))

    in_maps = []
    for c in range(N_CORES):
        e0 = c * EPC
        in_maps.append({
            "xT": np.ascontiguousarray(xf_a[gidx[c]].T),
            "w1": W1t[e0:e0 + EPC],
            "w2": W2t[e0:e0 + EPC],
            "b1": b1t[:, e0:e0 + EPC],
            "b2": b2t[:, e0:e0 + EPC],
        })

    nc = _get_nc(TCH, CW, dt_w, dt_a)
    trace = os.environ.get("MOE_TRACE", "0") == "1"
    res = bass_utils.run_bass_kernel_spmd(
        nc, in_maps, core_ids=list(range(N_CORES)), trace=trace,
    )
    LAST_EXEC_TIME_NS = res.exec_time_ns

    # ---- combine: gather each (token, k) contribution, weight, and sum
    Ystack = np.stack([res.results[c]["yT"].T for c in range(N_CORES)])
    contrib = Ystack[core, col]              # [k*T, D] (sorted order)
    inv = np.empty_like(order)
    inv[order] = np.arange(TOP_K * T)
    contrib = contrib[inv].reshape(T, TOP_K, D)
    y = (contrib * w[:, :, None]).sum(1).astype(np.float32)
    return y.reshape(orig_shape)



# revision 5
# speedup vs baseline: 1.0165x; 1.0165x over previous
"""MoE MLP (E=32 experts, top-2, D=H=1024) on 8 Trainium2 NeuronCores.

Strategy (expert parallel, per sharding hint):
  * Host computes the (tiny) gate: softmax(x @ Wg), top-2, renormalized
    weights, then dispatches tokens per expert into per-slot token blocks.
    Experts are rank-sorted by token count and assigned round-robin across
    cores so that slot j on every core has the same (static) capacity
    S[j] = max count in rank group j -- this cuts capacity padding from
    ~19% (uniform max capacity) to ~4%.
  * Each of the 8 cores owns 4 expert slots (weights gathered per the
    assignment) and computes GELU(x W1 + b1) W2 + b2 per slot.
  * Host combines with the top-2 gate weights (scatter-add).

Device kernel notes (from NTFF trace analysis of the previous version):
  * All weight/activation HBM tensors are host-pre-tiled PARTITION-MAJOR
    ([p=128, ...contiguous...]) so every DMA is ~128 descriptors of
    2-16KB each instead of thousands of 0.6-2KB ones.  This collapses
    descriptor-generation and issue latency (first weight chunk lands
    a few us after kernel start instead of ~9us).
  * Weight DMA triggers alternate between the sync and vector queues so
    two DMA rings stream weights concurrently.
  * y is written out bf16 in 2-column-tile pieces as soon as each pair of
    PSUM tiles retires, so the post-compute tail is one small DMA.
  * A short chain of warmup matmuls (no DMA deps) ramps the PE p-state
    (0.65 -> 2.4 GHz takes ~3us of continuous PE activity) while the
    first weight chunk lands, and feeds directly into the real matmuls
    so the clock never drops back.
"""

import os
import sys
import numpy as np

for _p in ("/root/.axon_site/_ro/trn_rl_repo", "/opt/trn_rl_repo"):
    if _p not in sys.path and os.path.isdir(_p):
        sys.path.append(_p)

E, D, H = 32, 1024, 1024
TOP_K = 2
N_CORES = 8
EPC = E // N_CORES  # expert slots per core
ND = D // 128       # d 128-tiles
NH = H // 128       # h 128-tiles

DT_W = os.environ.get("MOE_DT_W", "bfloat16")
DT_A = os.environ.get("MOE_DT_A", "bfloat16")
DT_Y = os.environ.get("MOE_DT_Y", "bfloat16")
N_WARMUP_MM = int(os.environ.get("MOE_WARMUP", "12"))
WU_COLS = int(os.environ.get("MOE_WU_COLS", "256"))
MAX_CW = 512  # PSUM bank limit: 512 f32 per partition

LAST_EXEC_TIME_NS = None

_NC_CACHE = {}


def _chunks(total, n):
    """Split `total` into n near-equal integer chunks."""
    base, rem = divmod(total, n)
    return [base + (1 if i < rem else 0) for i in range(n)]


def _build_nc(sizes, dt_w_name, dt_a_name, dt_y_name):
    import concourse.bass as bass  # noqa: F401
    import concourse.tile as tile
    from concourse import bacc, mybir
    from contextlib import ExitStack

    f32 = mybir.dt.float32
    dt_w = getattr(mybir.dt, dt_w_name)
    dt_a = getattr(mybir.dt, dt_a_name)
    dt_y = getattr(mybir.dt, dt_y_name)
    TT = sum(sizes)
    SMAX = max(sizes)

    # per-slot column-chunking (only kicks in if a slot exceeds one PSUM bank)
    slot_chunks = [_chunks(s, -(-s // MAX_CW)) for s in sizes]
    CWMAX = max(max(ch) for ch in slot_chunks)

    nc = bacc.Bacc(
        "TRN2",
        target_bir_lowering=False,
        debug=False,
        enable_asserts=False,
        num_devices=N_CORES,
    )
    # partition-major layouts: per-partition rows are contiguous in HBM
    xT = nc.dram_tensor("xT", [128, ND * TT], dt_a, kind="ExternalInput").ap()
    w1 = nc.dram_tensor("w1", [EPC, 128, NH, ND, 128], dt_w, kind="ExternalInput").ap()
    w2 = nc.dram_tensor("w2", [EPC, 128, ND, NH, 128], dt_w, kind="ExternalInput").ap()
    b1 = nc.dram_tensor("b1", [128, EPC, NH], f32, kind="ExternalInput").ap()
    b2 = nc.dram_tensor("b2", [128, EPC, ND], f32, kind="ExternalInput").ap()
    yT = nc.dram_tensor("yT", [128, ND * TT], dt_y, kind="ExternalOutput").ap()

    # W1 ht-chunks: small first chunk on slot 0 so compute starts early
    w1_chunk_plan = [[1, 3, 4]] + [[4, 4]] * (EPC - 1)
    w2_chunk_plan = [[4, 4]] * EPC

    xbufs = 1 if ND * SMAX * mybir.dt.size(dt_a) > 50 * 1024 else 2

    with tile.TileContext(nc) as tc, ExitStack() as ctx:
        wpool = ctx.enter_context(tc.tile_pool(name="w", bufs=4))
        xpool = ctx.enter_context(tc.tile_pool(name="x", bufs=xbufs))
        hpool = ctx.enter_context(tc.tile_pool(name="h", bufs=2 * NH))
        ypool = ctx.enter_context(tc.tile_pool(name="y", bufs=2))
        bpool = ctx.enter_context(tc.tile_pool(name="b", bufs=1))
        pp1 = ctx.enter_context(tc.tile_pool(name="ps1", bufs=3, space="PSUM"))
        pp2 = ctx.enter_context(tc.tile_pool(name="ps2", bufs=3, space="PSUM"))
        ppw = ctx.enter_context(tc.tile_pool(name="psw", bufs=1, space="PSUM"))

        gelu = mybir.ActivationFunctionType.Gelu

        # --- warmup: ramp the PE p-state while the first weight DMA lands.
        if N_WARMUP_MM:
            wu = bpool.tile([128, max(WU_COLS, 128)], mybir.dt.bfloat16, tag="wu")
            nc.vector.memset(wu[:], 0.0)
            wups = ppw.tile([128, max(WU_COLS, 128)], f32, tag="psw")
            for i in range(N_WARMUP_MM):
                nc.tensor.matmul(wups[:, :WU_COLS], wu[:, :128], wu[:, :WU_COLS],
                                 start=(i == 0), stop=(i == N_WARMUP_MM - 1))
            # pre-load the GELU table on the scalar engine off the critical path
            wug = bpool.tile([128, 1], f32, tag="wug")
            nc.scalar.activation(wug[:], wu[:, :1], gelu)

        # --- weight DMA triggers alternate sync/scalar queues (the two
        # HWDGE-capable engines); y writes go out via gpsimd
        dma_engines = [nc.sync, nc.scalar]
        eng_state = [0]

        def wdma(out_ap, in_ap):
            dma_engines[eng_state[0] % 2].dma_start(out=out_ap, in_=in_ap)
            eng_state[0] += 1

        xts, w1hs, w2hs = {}, {}, {}

        def issue_slot_dmas(j):
            S = sizes[j]
            base = ND * sum(sizes[:j])
            xt = xpool.tile([128, ND * SMAX], dt_a, tag="xt")
            nc.gpsimd.dma_start(out=xt[:, :ND * S], in_=xT[:, base:base + ND * S])
            xts[j] = xt
            w1h = []
            ht0 = 0
            for cht in w1_chunk_plan[j]:
                wt = wpool.tile([128, cht * ND * 128], dt_w, tag=f"w1_{cht}",
                                bufs=(4 if cht == 4 else 2))
                wdma(
                    wt[:].rearrange("p (ht dt hi) -> p ht dt hi", ht=cht, dt=ND),
                    w1[j, :, ht0:ht0 + cht],
                )
                w1h.append((ht0, cht, wt))
                ht0 += cht
            w1hs[j] = w1h
            w2h = []
            dt0 = 0
            for cdt in w2_chunk_plan[j]:
                wt = wpool.tile([128, cdt * NH * 128], dt_w, tag=f"w2_{cdt}",
                                bufs=(4 if cdt == 4 else 2))
                wdma(
                    wt[:].rearrange("p (dt ht di) -> p dt ht di", dt=cdt, ht=NH),
                    w2[j, :, dt0:dt0 + cdt],
                )
                w2h.append((dt0, cdt, wt))
                dt0 += cdt
            w2hs[j] = w2h

        issue_slot_dmas(0)
        b1_sb = bpool.tile([128, EPC * NH], f32, tag="b1")
        b2_sb = bpool.tile([128, EPC * ND], f32, tag="b2")
        nc.gpsimd.dma_start(
            out=b1_sb[:].rearrange("p (e ht) -> p e ht", e=EPC), in_=b1[:])
        nc.gpsimd.dma_start(
            out=b2_sb[:].rearrange("p (e dt) -> p e dt", e=EPC), in_=b2[:])

        for j in range(EPC):
            S = sizes[j]
            base = ND * sum(sizes[:j])
            xt, w1h, w2h = xts[j], w1hs[j], w2hs[j]
            if j + 1 < EPC and xbufs > 1:
                issue_slot_dmas(j + 1)

            # slot's yT block as [p, dt, t] for strided partial writes
            yslot = yT[:, base:base + ND * S].rearrange(
                "p (dt t) -> p dt t", dt=ND)

            c0 = 0
            for cw in slot_chunks[j]:
                # ---- layer 1: h[ht] = gelu(sum_dt W1[ht,dt]^T x[dt] + b1)
                hts = []
                for ht in range(NH):
                    ht0, cht, wt = next(w for w in w1h if w[0] <= ht < w[0] + w[1])
                    hoff = ((ht - ht0) * ND) * 128
                    ps = pp1.tile([128, CWMAX], f32, tag="ps1")
                    for dt_i in range(ND):
                        nc.tensor.matmul(
                            ps[:, :cw],
                            wt[:, hoff + dt_i * 128: hoff + (dt_i + 1) * 128],
                            xt[:, dt_i * S + c0: dt_i * S + c0 + cw],
                            start=(dt_i == 0),
                            stop=(dt_i == ND - 1),
                        )
                    hsb = hpool.tile([128, CWMAX], dt_a, tag="ht")
                    nc.scalar.activation(
                        hsb[:, :cw], ps[:, :cw], gelu,
                        bias=b1_sb[:, j * NH + ht: j * NH + ht + 1],
                    )
                    hts.append(hsb)
                # ---- layer 2: y[dt] = sum_ht W2[dt,ht]^T h[ht] + b2
                ysb = ypool.tile([128, ND * CWMAX], dt_y, tag="yt")
                for dt_i in range(ND):
                    dt0, cdt, wt = next(w for w in w2h if w[0] <= dt_i < w[0] + w[1])
                    doff = ((dt_i - dt0) * NH) * 128
                    ps2 = pp2.tile([128, CWMAX], f32, tag="ps2")
                    for ht in range(NH):
                        nc.tensor.matmul(
                            ps2[:, :cw],
                            wt[:, doff + ht * 128: doff + (ht + 1) * 128],
                            hts[ht][:, :cw],
                            start=(ht == 0),
                            stop=(ht == NH - 1),
                        )
                    nc.vector.tensor_scalar_add(
                        ysb[:, dt_i * cw:(dt_i + 1) * cw], ps2[:, :cw],
                        b2_sb[:, j * ND + dt_i: j * ND + dt_i + 1],
                    )
                    if dt_i % 2 == 1:
                        # stream out this pair of column tiles
                        nc.gpsimd.dma_start(
                            out=yslot[:, dt_i - 1:dt_i + 1, c0:c0 + cw],
                            in_=ysb[:, (dt_i - 1) * cw:(dt_i + 1) * cw]
                            .rearrange("p (two t) -> p two t", two=2),
                        )
                c0 += cw
            if j + 1 < EPC and xbufs == 1:
                issue_slot_dmas(j + 1)
    nc.compile()
    return nc


def _get_nc(sizes, dt_w, dt_a, dt_y):
    key = (tuple(sizes), dt_w, dt_a, dt_y)
    if key not in _NC_CACHE:
        _NC_CACHE[key] = _build_nc(list(sizes), dt_w, dt_a, dt_y)
    return _NC_CACHE[key]


def _np_dt(name):
    import ml_dtypes
    return np.dtype(getattr(ml_dtypes, name) if name != "float32" else np.float32)


def _route(xf, Wg):
    """Replicates the reference gate exactly in f32 numpy."""
    logits = xf @ Wg                                     # [T, E]
    m = logits.max(-1, keepdims=True)
    ex = np.exp(logits - m)
    scores = ex / ex.sum(-1, keepdims=True)
    idx = np.argsort(-scores, axis=1, kind="stable")[:, :TOP_K]  # [T, k]
    tw = np.take_along_axis(scores, idx, 1)
    m2 = tw.max(-1, keepdims=True)
    e2 = np.exp(tw - m2)
    w = (e2 / e2.sum(-1, keepdims=True)).astype(np.float32)
    return idx.astype(np.int64), w


def kernel(x, Wg, W1, b1, W2, b2):
    global LAST_EXEC_TIME_NS
    from concourse import bass_utils

    dt_w, dt_a, dt_y = DT_W, DT_A, DT_Y
    orig_shape = x.shape
    x = np.asarray(x, dtype=np.float32)
    Wg = np.asarray(Wg, dtype=np.float32)
    W1 = np.asarray(W1, dtype=np.float32)
    b1 = np.asarray(b1, dtype=np.float32)
    W2 = np.asarray(W2, dtype=np.float32)
    b2 = np.asarray(b2, dtype=np.float32)
    xf = np.ascontiguousarray(x.reshape(-1, D))
    T = xf.shape[0]

    idx, w = _route(xf, Wg)

    # ---- slot assignment: rank experts by count, group ranks of N_CORES,
    # slot j capacity = max count in group j (aligned up to 4)
    counts = np.bincount(idx.reshape(-1), minlength=E)
    order_e = np.argsort(-counts, kind="stable")         # expert ids by rank
    exp_core = np.empty(E, np.int64)
    exp_slot = np.empty(E, np.int64)
    sizes = []
    for j in range(EPC):
        grp = order_e[j * N_CORES:(j + 1) * N_CORES]
        exp_core[grp] = np.arange(N_CORES)
        exp_slot[grp] = j
        sizes.append(max(4, int(-(-int(counts[grp].max()) // 4) * 4)))
    TT = sum(sizes)
    slot_off = np.concatenate([[0], np.cumsum(sizes)])   # token offsets per slot

    # ---- dispatch: sort assignments by expert, position within expert
    flat_e = idx.reshape(-1)                 # [T*k]
    flat_t = np.repeat(np.arange(T), TOP_K)
    sorder = np.argsort(flat_e, kind="stable")
    starts = np.zeros(E + 1, np.int64)
    starts[1:] = np.cumsum(counts)
    se = flat_e[sorder]
    pos = np.arange(TOP_K * T) - starts[se]
    core = exp_core[se]
    slot = exp_slot[se]
    tok = flat_t[sorder]

    # token id occupying each (core, slot, pos); padding -> token 0
    gidx = np.zeros((N_CORES, TT), np.int64)
    for c in range(N_CORES):
        msel = core == c
        gidx[c, slot_off[slot[msel]] + pos[msel]] = tok[msel]

    np_w = _np_dt(dt_w)
    np_a = _np_dt(dt_a)
    xf_a = xf.astype(np_a, copy=False)
    # pre-tile weights partition-major:
    #   w1[e] = [p(=d_in%128), ht, dt(=d_in//128), hi]
    #   w2[e] = [p(=h_in%128), dt, ht(=h_in//128), di]
    W1t = np.ascontiguousarray(
        W1.reshape(E, ND, 128, NH, 128).transpose(0, 2, 3, 1, 4).astype(np_w, copy=False))
    W2t = np.ascontiguousarray(
        W2.reshape(E, NH, 128, ND, 128).transpose(0, 2, 3, 1, 4).astype(np_w, copy=False))
    b1t = np.ascontiguousarray(b1.reshape(E, NH, 128).transpose(2, 0, 1))
    b2t = np.ascontiguousarray(b2.reshape(E, ND, 128).transpose(2, 0, 1))

    in_maps = []
    for c in range(N_CORES):
        sl_experts = np.array(
            [order_e[j * N_CORES + c] for j in range(EPC)], np.int64)
        # xT: [128, ND*TT]; slot block = [128, ND, S_j] = x[tok, dt*128+p]
        xTc = np.zeros((128, ND * TT), np_a)
        for j in range(EPC):
            S = sizes[j]
            blk = xf_a[gidx[c, slot_off[j]:slot_off[j] + S]]  # [S, D]
            xTc[:, ND * slot_off[j]: ND * slot_off[j] + ND * S] = (
                blk.reshape(S, ND, 128).transpose(2, 1, 0).reshape(128, ND * S))
        in_maps.append({
            "xT": np.ascontiguousarray(xTc),
            "w1": W1t[sl_experts],
            "w2": W2t[sl_experts],
            "b1": np.ascontiguousarray(b1t[:, sl_experts]),
            "b2": np.ascontiguousarray(b2t[:, sl_experts]),
        })

    nc = _get_nc(sizes, dt_w, dt_a, dt_y)
    trace = os.environ.get("MOE_TRACE", "0") == "1"
    res = bass_utils.run_bass_kernel_spmd(
        nc, in_maps, core_ids=list(range(N_CORES)), trace=trace,
    )
    LAST_EXEC_TIME_NS = res.exec_time_ns

    # ---- combine: y[t] += w * yT[core][:, slot block][:, :, pos]
    Y = np.zeros((N_CORES, TT, D), np.float32)
    for c in range(N_CORES):
        yTc = np.asarray(res.results[c]["yT"], dtype=np.float32)
        for j in range(EPC):
            S = sizes[j]
            blk = yTc[:, ND * slot_off[j]: ND * slot_off[j] + ND * S]
            Y[c, slot_off[j]:slot_off[j] + S] = (
                blk.reshape(128, ND, S).transpose(2, 1, 0).reshape(S, D))

    contrib = Y[core, slot_off[slot] + pos]  # [T*k, D] in sorted order
    inv = np.empty_like(sorder)
    inv[sorder] = np.arange(TOP_K * T)
    contrib = contrib[inv].reshape(T, TOP_K, D)
    y = (contrib * w[:, :, None]).sum(1).astype(np.float32)
    return y.reshape(orig_shape)
